# revision 40
# baseline (speedup 1.0000x reference)
"""AttentionBlock (GroupNorm + single-head 4096x4096 attention + residual) on 8 trn2 cores.

Sharding: core = 2*b + h. Data-parallel over batch (B=4), sequence-parallel over
query rows (2 halves of 2048). Each core receives its batch's x transposed to
[C, N] with token columns rotated so the core's own query tokens are columns
0..2047 (keeps the SPMD program uniform). K/V are computed for all 4096 tokens
on both cores of a pair (duplicated ~17% of FLOPs, no collectives).

All heavy matmuls run in float32r (TF32-like, full PE throughput, ~1.5e-4 rel).

Memory plan (per core, SBUF 224KB/partition):
  resident: K^T (8MB f32r), Q^T own-half (4MB f32r), x^T own-half (4MB f32),
            weights (4MB f32r during QKV, wp 1MB after)
  streamed: V round-trips through DRAM (written in stage B, prefetch-streamed
            per key tile during attention) — trades idle-DMA bandwidth for the
            SBUF needed to keep x/Q resident.
"""

import os

import numpy as np
from contextlib import ExitStack

import concourse.bacc as bacc
import concourse.bass as bass
import concourse.bass_isa as bass_isa
import concourse.mybir as mybir
import concourse.tile as tile
from concourse.bass_utils import run_bass_kernel_spmd

F32 = mybir.dt.float32
F32R = mybir.dt.float32r
AF = mybir.ActivationFunctionType
OP = mybir.AluOpType

B, HH, WW, C = 4, 64, 64, 512
NTOK = HH * WW          # 4096 tokens per batch
NOWN = NTOK // 2        # 2048 own query tokens per core
GROUPS = 32
CG = C // GROUPS        # 16 channels per group
EPS = 1e-5
CT = C // 128           # 4 channel tiles
QTOK = 1024             # token quarter for stage B
NQ = NTOK // QTOK       # 4 quarters
JT = NTOK // 128        # 32 key token tiles
IB = NOWN // 512        # 4 query i-blocks per core
SCALE = float(C) ** -0.5

_CACHE = {}


def _build_nc():
    if "nc" in _CACHE:
        return _CACHE["nc"]

    nc = bacc.Bacc(trn_type="TRN2")

    xT = nc.dram_tensor("xT", [C, NTOK], F32, kind="ExternalInput")
    w_ext = {
        n: nc.dram_tensor(n, [C, C], F32, kind="ExternalInput")
        for n in ("wq", "wk", "wv", "wp")
    }
    b_ext = {
        n: nc.dram_tensor(n, [C], F32, kind="ExternalInput")
        for n in ("bq", "bk", "bv", "bp")
    }
    gamma_ext = nc.dram_tensor("gamma", [C], F32, kind="ExternalInput")
    beta_ext = nc.dram_tensor("beta", [C], F32, kind="ExternalInput")
    gsel_ext = nc.dram_tensor("gsel", [128, 8], F32, kind="ExternalInput")
    gselT_ext = nc.dram_tensor("gselT", [8, 128], F32, kind="ExternalInput")
    yT_ext = nc.dram_tensor("yT", [C, NOWN], F32, kind="ExternalOutput")

    with ExitStack() as ctx:
        tc = ctx.enter_context(tile.TileContext(nc))

        # ---- persistent pools ------------------------------------------------
        smalls = ctx.enter_context(tc.tile_pool(name="smalls", bufs=1))
        gnp = ctx.enter_context(tc.tile_pool(name="gnp", bufs=2))
        ktp = ctx.enter_context(tc.tile_pool(name="ktp", bufs=1))
        qres = ctx.enter_context(tc.tile_pool(name="qres", bufs=1))
        xown = ctx.enter_context(tc.tile_pool(name="xown", bufs=1))
        wpp = ctx.enter_context(tc.tile_pool(name="wpp", bufs=1))
        vdram = ctx.enter_context(tc.tile_pool(name="vdram", bufs=1, space="DRAM"))

        psA = ctx.enter_context(tc.tile_pool(name="psA", bufs=3, space="PSUM"))
        psO = ctx.enter_context(tc.tile_pool(name="psO", bufs=4, space="PSUM"))
        psM = ctx.enter_context(tc.tile_pool(name="psM", bufs=1, space="PSUM"))

        # ---- small constants -------------------------------------------------
        ones1_f = smalls.tile([1, 128], F32, tag="ones1_f")
        nc.vector.memset(ones1_f, 1.0)
        ones_f = smalls.tile([128, 1], F32, tag="ones_f")
        nc.vector.memset(ones_f, 1.0)
        ones_r = smalls.tile([128, 1], F32R, tag="ones_r")
        nc.vector.tensor_copy(ones_r[:], ones_f[:])
        eps_row = smalls.tile([8, 1], F32, tag="eps_row")
        nc.vector.memset(eps_row, EPS)

        gsel_sb = smalls.tile([128, 8], F32, tag="gsel")
        nc.sync.dma_start(gsel_sb[:], gsel_ext[:])
        gselT_sb = smalls.tile([8, 128], F32, tag="gselT")
        nc.sync.dma_start(gselT_sb[:], gselT_ext[:])

        def col_tiles(ext, tag):
            v = ext.rearrange("(t p) -> t p", p=128)
            ts = []
            for t in range(CT):
                s = smalls.tile([128, 1], F32, tag=f"{tag}{t}")
                nc.sync.dma_start(s[:], v[t][:, None])
                ts.append(s)
            return ts

        gamma_t = col_tiles(gamma_ext, "gamma")
        beta_t = col_tiles(beta_ext, "beta")
        bq_t = col_tiles(b_ext["bq"], "bq")
        bk_t = col_tiles(b_ext["bk"], "bk")
        bp_t = col_tiles(b_ext["bp"], "bp")

        # bv broadcast across partitions via ones matmul
        bv_row = smalls.tile([1, C], F32, tag="bv_row")
        nc.sync.dma_start(bv_row[:], b_ext["bv"].rearrange("c -> () c"))
        bv_ps = psM.tile([128, C], F32, tag="psM")
        nc.tensor.matmul(bv_ps[:], ones1_f[:], bv_row[:], start=True, stop=True)
        bv_bc = smalls.tile([128, C], F32, tag="bv_bc")
        nc.vector.tensor_copy(bv_bc[:], bv_ps[:])

        wqkv_ctx = ExitStack()
        wqkvp = wqkv_ctx.enter_context(tc.tile_pool(name="wqkv", bufs=1))
        w_r = {}

        KT = [ktp.tile([128, NTOK], F32R, tag=f"kt{t}", name=f"kt{t}") for t in range(CT)]
        # own-half x tiles stay resident: (q in 0..1) x (channel tile)
        xo = {}
        for q in range(2):
            for t in range(CT):
                xo[q, t] = xown.tile(
                    [128, QTOK], F32, tag=f"xo{q}{t}", name=f"xo{q}{t}"
                )
        # resident Q^T: (quarter, co, 512-chunk)
        qtiles = {}
        for q in range(2):
            for co in range(CT):
                for nch in range(QTOK // 512):
                    qtiles[q, co, nch] = qres.tile(
                        [128, 512], F32R, tag=f"q{q}{co}{nch}", name=f"q{q}{co}{nch}"
                    )
        v_dram = vdram.tile([NTOK, C], F32R)
        gst_in = vdram.tile([8, 2 * CT], F32, name="gst_in")
        gst_out = vdram.tile([8, 2 * CT], F32, name="gst_out")

        # ---- stage A: groupnorm statistics (one streaming pass over xT) ------
        # x DMAs are the critical path at kernel start (HBM contended by all 8
        # cores) — issue them before the weight loads.
        with (
            tc.tile_pool(name="xq", bufs=3) as xqp,
            tc.tile_pool(name="wst", bufs=2) as wstp,
            nc.named_scope("stats"),
        ):
            # stats over the OWN half only (resident tiles); the other half's
            # contribution arrives via a tiny pair AllReduce of the group sums.
            stats_t = [
                gnp.tile([128, 4, 6], F32, tag=f"stats{t}", name=f"stats{t}")
                for t in range(CT)
            ]
            for q in range(2):
                for t in range(CT):
                    xq_t = xo[q, t]
                    nc.sync.dma_start(
                        xq_t[:], xT[t * 128 : (t + 1) * 128, q * QTOK : (q + 1) * QTOK]
                    )
                    nc.vector.bn_stats(stats_t[t][:, 2 * q, :], xq_t[:, 0:512])
                    nc.vector.bn_stats(stats_t[t][:, 2 * q + 1, :], xq_t[:, 512:1024])

            # weights land during the stats compute
            for n in ("wq", "wk", "wv", "wp"):
                w_r[n] = []
                for ci in range(CT):
                    st = wstp.tile([128, C], F32, tag="wst")
                    nc.sync.dma_start(st[:], w_ext[n][ci * 128 : (ci + 1) * 128, :])
                    pool = wpp if n == "wp" else wqkvp
                    wr = pool.tile([128, C], F32R, tag=f"{n}{ci}")
                    nc.vector.tensor_copy(wr[:], st[:])
                    w_r[n].append(wr)
            wp_r = w_r["wp"]

            packed = gnp.tile([128, 2 * CT], F32, tag="packed")
            for t in range(CT):
                mv = gnp.tile([128, 2], F32, tag="mv")
                nc.vector.bn_aggr(mv[:], stats_t[t][:])
                nc.vector.tensor_copy(packed[:, 2 * t : 2 * t + 1], mv[:, 0:1])
                tmp = gnp.tile([128, 1], F32, tag="tmp")
                nc.vector.tensor_mul(tmp[:], mv[:, 0:1], mv[:, 0:1])
                nc.vector.tensor_add(
                    packed[:, 2 * t + 1 : 2 * t + 2], mv[:, 1:2], tmp[:]
                )

            g_ps = psM.tile([8, 2 * CT], F32, tag="psM")
            nc.tensor.matmul(g_ps[:], gsel_sb[:], packed[:], start=True, stop=True)
            g_sb = gnp.tile([8, 2 * CT], F32, tag="g_sb")
            nc.vector.tensor_copy(g_sb[:], g_ps[:])
            nc.sync.dma_start(gst_in[:], g_sb[:])
            nc.gpsimd.collective_compute(
                "AllReduce",
                OP.add,
                replica_groups=[[0, 1], [2, 3], [4, 5], [6, 7]],
                ins=[gst_in.opt()],
                outs=[gst_out.opt()],
            )
            g2_sb = gnp.tile([8, 2 * CT], F32, tag="g2_sb")
            nc.sync.dma_start(g2_sb[:], gst_out[:])
            stat2 = gnp.tile([8, 2 * CT], F32, tag="stat2")
            nc.vector.tensor_scalar_mul(stat2[:], g2_sb[:], 1.0 / (2 * CG))
            s2v = stat2.rearrange("g (t two) -> g t two", two=2)
            mu_v = s2v[:, :, 0]
            e2_v = s2v[:, :, 1]
            musq = gnp.tile([8, CT], F32, tag="musq")
            nc.vector.tensor_mul(musq[:], mu_v, mu_v)
            var = gnp.tile([8, CT], F32, tag="var")
            nc.vector.tensor_sub(var[:], e2_v, musq[:])
            sqv = gnp.tile([8, CT], F32, tag="sqv")
            nc.scalar.activation(sqv[:], var[:], AF.Sqrt, bias=eps_row[:], scale=1.0)
            rstd = gnp.tile([8, CT], F32, tag="rstd")
            nc.vector.reciprocal(rstd[:], sqv[:])

            scale_t, shift_t = [], []
            for t in range(CT):
                cat2 = gnp.tile([8, 2], F32, tag="cat2")
                nc.vector.tensor_copy(cat2[:, 0:1], mu_v[:, t : t + 1])
                nc.vector.tensor_copy(cat2[:, 1:2], rstd[:, t : t + 1])
                bc_ps = psM.tile([128, 2], F32, tag="psM")
                nc.tensor.matmul(bc_ps[:], gselT_sb[:], cat2[:], start=True, stop=True)
                sc = gnp.tile([128, 1], F32, tag=f"scale{t}")
                nc.vector.tensor_mul(sc[:], bc_ps[:, 1:2], gamma_t[t][:])
                tmp2 = gnp.tile([128, 1], F32, tag="tmp2")
                nc.vector.tensor_mul(tmp2[:], bc_ps[:, 0:1], sc[:])
                sh = gnp.tile([128, 1], F32, tag=f"shift{t}")
                nc.vector.tensor_sub(sh[:], beta_t[t][:], tmp2[:])
                scale_t.append(sc)
                shift_t.append(sh)

        # ---- stage B: normalize + QKV projections ----------------------------
        with (
            tc.tile_pool(name="xq2", bufs=2) as xqp2,
            tc.tile_pool(name="xnr", bufs=6) as xnrp,
            tc.tile_pool(name="vb", bufs=2) as vbp,
            nc.named_scope("qkv"),
        ):
            for q in range(NQ):
                xn = []
                for t in range(CT):
                    if q < 2:
                        src = xo[q, t]
                    else:
                        src = xqp2.tile([128, QTOK], F32, tag="xq2")
                        nc.sync.dma_start(
                            src[:],
                            xT[t * 128 : (t + 1) * 128, q * QTOK : (q + 1) * QTOK],
                        )
                    xn_t = xnrp.tile([128, QTOK], F32R, tag="xnr")
                    nc.vector.tensor_scalar(
                        out=xn_t[:],
                        in0=src[:],
                        scalar1=scale_t[t][:],
                        scalar2=shift_t[t][:],
                        op0=OP.mult,
                        op1=OP.add,
                    )
                    xn.append(xn_t)

                # K^T (all quarters) and Q^T (own-half quarters, kept resident)
                for name, dst_bias in (("wk", bk_t), ("wq", bq_t)):
                    if name == "wq" and q >= 2:
                        continue
                    for co in range(CT):
                        for nch in range(QTOK // 512):
                            ps = psO.tile([128, 512], F32, tag="psO")
                            for ci in range(CT):
                                nc.tensor.matmul(
                                    ps[:],
                                    w_r[name][ci][:, co * 128 : (co + 1) * 128],
                                    xn[ci][:, nch * 512 : (nch + 1) * 512],
                                    start=(ci == 0),
                                    stop=(ci == CT - 1),
                                )
                            if name == "wk":
                                dst = KT[co][
                                    :,
                                    q * QTOK + nch * 512 : q * QTOK + (nch + 1) * 512,
                                ]
                            else:
                                dst = qtiles[q, co, nch][:]
                            nc.scalar.activation(
                                dst, ps[:], AF.Identity, bias=dst_bias[co][:], scale=1.0
                            )

                # V natural [tok, C], written out to DRAM (streamed back in C)
                for jt in range(QTOK // 128):
                    j = q * (QTOK // 128) + jt
                    ps = psA.tile([128, 512], F32, tag="psA")
                    for ci in range(CT):
                        nc.tensor.matmul(
                            ps[:],
                            xn[ci][:, jt * 128 : (jt + 1) * 128],
                            w_r["wv"][ci][:],
                            start=(ci == 0),
                            stop=(ci == CT - 1),
                        )
                    vb_t = vbp.tile([128, C], F32R, tag="vb")
                    nc.vector.tensor_add(vb_t[:], ps[:], bv_bc[:])
                    nc.sync.dma_start(v_dram[j * 128 : (j + 1) * 128, :], vb_t[:])

        wqkv_ctx.close()

        # ---- stage C: attention + projection ---------------------------------
        with (
            tc.tile_pool(name="vstream", bufs=10) as vsp,
            tc.tile_pool(name="pt", bufs=3) as ptp,
            tc.tile_pool(name="osb", bufs=4) as osbp,
            tc.tile_pool(name="ysb", bufs=4) as ysbp,
            tc.tile_pool(name="racc", bufs=2) as raccp,
            tc.tile_pool(name="rsb", bufs=2) as rsbp,
            nc.named_scope("attn"),
        ):
            for ib in range(IB):
                i0 = ib * 512
                qq, nch = ib // 2, ib % 2
                qblk = [qtiles[qq, co, nch] for co in range(CT)]

                psO_t = [
                    psO.tile([128, 512], F32, tag="psO", name=f"psO_{ib}_{i}")
                    for i in range(CT)
                ]
                racc = raccp.tile([128, 512], F32R, tag="racc")

                def emit_o(j, pt_t, v_t):
                    if j == 0:
                        nc.vector.tensor_copy(racc[:], pt_t[:])
                    else:
                        nc.vector.tensor_add(racc[:], racc[:], pt_t[:])
                    for ct in range(CT):
                        nc.tensor.matmul(
                            psO_t[ct][:],
                            v_t[:, ct * 128 : (ct + 1) * 128],
                            pt_t[:],
                            start=(j == 0),
                            stop=(j == JT - 1),
                        )

                def emit_r_chain():
                    psr = psM.tile([1, 512], F32, tag="psM")
                    nc.tensor.matmul(psr[:], ones_r[:], racc[:], start=True, stop=True)
                    rinv = rsbp.tile([1, 512], F32, tag="rinv")
                    nc.vector.reciprocal(rinv[:], psr[:])
                    rb_ps = psM.tile([128, 512], F32, tag="psM")
                    nc.tensor.matmul(
                        rb_ps[:], ones1_f[:], rinv[:], start=True, stop=True
                    )
                    rb_sb = rsbp.tile([128, 512], F32, tag="rb_sb")
                    nc.vector.tensor_copy(rb_sb[:], rb_ps[:])
                    return rb_sb

                prev = None  # (j, pt_tile, v_tile)
                for j in range(JT):
                    v_t = vsp.tile([128, C], F32R, tag="vstream")
                    nc.sync.dma_start(v_t[:], v_dram[j * 128 : (j + 1) * 128, :])
                    psS = psA.tile([128, 512], F32, tag="psA")
                    for ci in range(CT):
                        nc.tensor.matmul(
                            psS[:],
                            KT[ci][:, j * 128 : (j + 1) * 128],
                            qblk[ci][:],
                            start=(ci == 0),
                            stop=(ci == CT - 1),
                        )
                    if prev is not None:
                        emit_o(*prev)
                    pt_t = ptp.tile([128, 512], F32R, tag="pt")
                    nc.scalar.activation(pt_t[:], psS[:], AF.Exp, scale=SCALE)
                    prev = (j, pt_t, v_t)
                emit_o(*prev)

                # project the UNNORMALIZED O; fold the softmax denominator into
                # the final y scaling so the r-chain overlaps the proj matmuls.
                rb_sb = emit_r_chain()

                osb = []
                for ct in range(CT):
                    o_t = osbp.tile([128, 512], F32R, tag="osb")
                    nc.vector.tensor_copy(o_t[:], psO_t[ct][:])
                    osb.append(o_t)

                for co in range(CT):
                    psY = psA.tile([128, 512], F32, tag="psA")
                    for ci in range(CT):
                        nc.tensor.matmul(
                            psY[:],
                            wp_r[ci][:, co * 128 : (co + 1) * 128],
                            osb[ci][:],
                            start=(ci == 0),
                            stop=(ci == CT - 1),
                        )
                    xr = xo[qq, co][:, nch * 512 : (nch + 1) * 512]
                    y1_t = ysbp.tile([128, 512], F32, tag="y1sb")
                    nc.vector.tensor_mul(y1_t[:], psY[:], rb_sb[:])
                    y_t = ysbp.tile([128, 512], F32, tag="ysb")
                    nc.vector.scalar_tensor_tensor(
                        out=y_t[:],
                        in0=y1_t[:],
                        scalar=bp_t[co][:],
                        in1=xr,
                        op0=OP.add,
                        op1=OP.add,
                    )
                    nc.sync.dma_start(
                        yT_ext[co * 128 : (co + 1) * 128, i0 : i0 + 512], y_t[:]
                    )

    nc.compile()
    _CACHE["nc"] = nc
    return nc


def kernel(x, gamma, beta, wq, bq, wk, bk, wv, bv, wp, bp):
    nc = _build_nc()

    x = np.asarray(x, dtype=np.float32)
    gsel = np.zeros((128, 8), np.float32)
    for p in range(128):
        gsel[p, p // CG % 8] = 1.0
    gselT = np.ascontiguousarray(gsel.T)

    shared = {
        "wq": np.asarray(wq, np.float32),
        "wk": np.asarray(wk, np.float32),
        "wv": np.asarray(wv, np.float32),
        "wp": np.asarray(wp, np.float32),
        "bq": np.asarray(bq, np.float32),
        "bk": np.asarray(bk, np.float32),
        "bv": np.asarray(bv, np.float32),
        "bp": np.asarray(bp, np.float32),
        "gamma": np.asarray(gamma, np.float32),
        "beta": np.asarray(beta, np.float32),
        "gsel": gsel,
        "gselT": gselT,
    }

    in_maps = []
    for core in range(8):
        b, h = core // 2, core % 2
        xT_b = np.ascontiguousarray(x[b].reshape(NTOK, C).T)  # [C, NTOK]
        if h == 1:
            xT_b = np.ascontiguousarray(
                np.concatenate([xT_b[:, NOWN:], xT_b[:, :NOWN]], axis=1)
            )
        in_maps.append({"xT": xT_b, **shared})

    res = run_bass_kernel_spmd(nc, in_maps, core_ids=list(range(8)))

    y = np.empty((B, NTOK, C), np.float32)
    for core in range(8):
        b, h = core // 2, core % 2
        yT = res.results[core]["yT"]  # [C, NOWN]
        y[b, h * NOWN : (h + 1) * NOWN, :] = yT.T
    return y.reshape(B, HH, WW, C)
